# revision 51
# baseline (speedup 1.0000x reference)
"""Trainium2 Bass kernel: segment-mean + pairwise-diff edge MLP (restructured).

Reference (per batch row b):
  seg = cumsum(ids == 3); valid = ~sep
  means[n] = mean of features[s] over tokens with seg==n & valid (n < 8)
  diff[i,j] = means[i] - means[j]                              # [8,8,H]
  out[i,j]  = relu(relu(diff @ W1 + b1) @ Wm + bm) @ W2 + b2   # [8,8,150]

Key observations driving this version (vs the v1 full-stream kernel):
  1. Only tokens BEFORE the 8th separator contribute (seg < 8). With
     P(sep)=1/8 that is ~60 of 1024 tokens per row -> ~94% of the feature
     HBM traffic in v1 was multiplied by an all-zero one-hot. The host
     packs exactly the contributing tokens (128-token chunks, zero-pad
     tail) -> ~1.6 MB instead of 25 MB per core.
  2. The segment-mean AND the pairwise diff fold into one host-built
     matrix: ohE4[t,(r,i,j)] = oh[t,i]/c_i - oh[t,j]/c_j, so a single
     accumulating matmul per (chunk, h-slice) produces diffT directly in
     PSUM (feats chunk is the stationary operand):
         diffT[h,(r,i,j)] = sum_t feats[t,h] * ohE4[t,(r,i,j)]
     No means stage, no eviction of means, no transpose stage.
  3. Antisymmetry: diff[j,i] = -diff[i,j] and relu breaks it only AFTER
     mm1's product: y = W1^T diffT computed for i<j only (28 of 64
     pairs); h1+ = relu(+y+b1), h1- = relu(-y+b1) reuse the one product.
     mm1/diffT stream width drops 64->28 per row pair. The diagonal
     out[i,i] = f(0) is input-independent -> computed on host in fp32.

Distribution: 128 batch rows sorted by token count, snake-dealt into 16
bins of 8 rows (8 cores x 2 super-groups); per SG tokens are packed
densely into ceil(T/128) chunks of [128 tok, 768] bf16.

Device program per core (2 super-groups, SG = 8 rows = 224 diffT cols):
  diffT: for hc in 6: for chunk: matmul(dp[hc] (+)= featsT_chunk[hc] @
         ohE4_chunk), N=224 moving, feats stationary. hc-major so each
         2-hc PSUM bank is cast (fp32->bf16) while later hc still
         accumulate; casts alternate vector/scalar.
  mm1:   y[ci] = W1^T @ diffT (accumulate 6 h-chunks), ci = c-split
         128+22; h1 = [relu(y+b1) | relu(-y+b1)] -> [csz, 448] bf16.
  mm2:   h2 = relu(Wm^T h1 + bm), k-split 128+22, N=448.
  mm3:   out = h2^T-slices @ [W2 | b2-row] -> 4 x [112, 150] fp32 per SG
         (the b2 row pairs with a constant-1 h2 row, so evictions are
         plain copies split vector/scalar); 2 out DMAs per SG with one
         contiguous 1200B line per partition.

Schedule notes (from perfetto):
  - exec is input-stream-bound at the start (~2MB critical bytes at
    ~358GB/s from body-start ~7.3us): ohT+bias on scalar, features on
    sync in (nc0-1, 1, nc1) pieces; the 384KB weight pack is emitted
    after diffT(SG0) so it never steals feature-stream bandwidth.
  - the HAM evaluates PE activity in free-running 3.413us epochs
    (4096cyc @1.2GHz) and only runs full-rate in an epoch if the prior
    ones were busy: 42 full-width dummy matmuls (zero-dep warmup) keep
    the array active from the moment the PE queue opens, and the 2-SG
    emission ladder [d0 d1 mm1_0 mm1_1 mm2_0 mm2_1 mm3_0 mm3_1] keeps
    PE gaps under the re-throttle window.
  - at full clock, ldweights(128 cols) overlaps the N=224 matmul stream
    almost perfectly (~96ns/pair).

PSUM banks: dp 2 + h1 2 + h2 1+1 + out/warm 2 = 8.
"""

import sys

import numpy as np
import ml_dtypes

if "/opt/trn_rl_repo" not in sys.path:
    sys.path.insert(0, "/opt/trn_rl_repo")

import concourse.bass as bass
import concourse.mybir as mybir
from concourse.bass import ds
from concourse.bass_utils import run_bass_kernel_spmd
from concourse.tile import TileContext



B, S, H, C = 128, 1024, 768, 150
NSEG = 8
SEP_ID = 3
NCORES = 8
NSG = 2                      # super-groups per core
NU = NSEG * (NSEG - 1) // 2  # 28 (i<j) pairs
RSG = B // (NCORES * NSG)    # 8 rows per super-group
NU8 = RSG * NU               # 224 diffT columns per SG
HC = H // 128                # 6 h-chunks
CC = ((0, 128), (128, 22))   # c-dim (150) split
NWARM = 42
# packed bf16 weight tensor column offsets
WPK_W1 = 0            # [128, HC*C] w1 h-major
WPK_WM0 = HC * C      # [128, C]
WPK_W20 = HC * C + C  # [128, C]
WPK_WM1 = HC * C + 2 * C   # [22, C]
WPK_W21E = HC * C + 3 * C  # [23, C] (row 22 = b2)
WPK_COLS = HC * C + 4 * C

F32 = mybir.dt.float32
BF16 = mybir.dt.bfloat16
NPBF16 = ml_dtypes.bfloat16

UI = np.array([i for i in range(NSEG) for j in range(i + 1, NSEG)])
UJ = np.array([j for i in range(NSEG) for j in range(i + 1, NSEG)])


def build_program(nc0, nc1):
    NCT = nc0 + nc1
    nc = bass.Bass("TRN2", target_bir_lowering=False, debug=False)

    # ALL bf16 inputs ride ONE per-partition-contiguous blob:
    #   [sg0 ohE4 | sg0 feats | sg1 ohE4 | sg1 feats | wpk]
    # -> 3 DMAs with ~8KB-per-partition descriptors (line-rate stream, one
    # wait per consumer) instead of 7 small ones. Every DMA costs a DMAHW
    # lane that ALL engines serially event-sem-wait on at the end barrier
    # (~115ns per entry on each queue), so DMA count is latency, not just
    # bandwidth. wpk = w1 | wm0 | w20 | wm1 | w21e (w21e rows 0-21 =
    # W2[128:], row 22 = b2: pairs with a constant-1 h2 row so mm3 adds b2
    # inside the matmul and evictions are plain copies).
    OH0 = 0
    FT0 = nc0 * NU8
    OH1 = nc0 * (NU8 + H)
    FT1 = OH1 + nc1 * NU8
    WPK0 = OH1 + nc1 * (NU8 + H)
    BLOBCOLS = WPK0 + WPK_COLS
    blob_d = nc.dram_tensor("blob", [128, BLOBCOLS], BF16,
                            kind="ExternalInput").ap()
    # packed fp32 biases: col0 = b1[:128], col1 = bm[:128],
    # col2 rows0-21 = b1[128:], col3 rows0-21 = bm[128:]
    bias_d = nc.dram_tensor("biasp", [128, 4], F32, kind="ExternalInput").ap()
    # one contiguous 2400B line per partition -> 112-descriptor out DMAs
    out_d = nc.dram_tensor("out", [NSG, 112, 4 * C], F32,
                           kind="ExternalOutput").ap()

    RELU = mybir.ActivationFunctionType.Relu
    COPY = mybir.ActivationFunctionType.Copy
    ADD = mybir.AluOpType.add
    MAX = mybir.AluOpType.max

    with TileContext(nc) as tc:
        with (
            tc.tile_pool(name="const", bufs=1) as constp,
            tc.tile_pool(name="diff", bufs=2) as diffp,
            tc.tile_pool(name="act", bufs=2) as actp,
            tc.tile_pool(name="osb", bufs=4) as osbp,
            tc.tile_pool(name="dps", bufs=2, space="PSUM") as dpsum,
            tc.tile_pool(name="h1ps", bufs=2, space="PSUM") as h1ps,
            tc.tile_pool(name="h2ps0", bufs=1, space="PSUM") as h2ps0,
            tc.tile_pool(name="h2ps1", bufs=1, space="PSUM") as h2ps1,
            tc.tile_pool(name="ops", bufs=2, space="PSUM") as opps,
        ):
            # one SBUF tile backs the whole blob; the per-SG DMAs land in
            # disjoint column ranges (subtile deps give each consumer a
            # single clean wait). SG0's slab on sync, SG1's on scalar, and
            # the wpk slab is emitted after diffT(SG0) so its 384KB never
            # steals bandwidth from the critical feature stream.
            in0 = constp.tile([128, BLOBCOLS], BF16, tag="in0")
            nc.sync.dma_start(out=in0[:, ds(0, OH1)], in_=blob_d[:, ds(0, OH1)])
            nc.scalar.dma_start(out=in0[:, ds(OH1, WPK0 - OH1)],
                                in_=blob_d[:, ds(OH1, WPK0 - OH1)])
            bias_sb = constp.tile([128, 4], F32, tag="biasp")
            nc.scalar.dma_start(out=bias_sb, in_=bias_d)
            w1_sb = in0[:, ds(WPK0 + WPK_W1, HC * C)]
            wm0_sb = in0[:, ds(WPK0 + WPK_WM0, C)]
            w20_sb = in0[:, ds(WPK0 + WPK_W20, C)]
            wm1_sb = in0[ds(0, 22), ds(WPK0 + WPK_WM1, C)]
            w21e_sb = in0[ds(0, 23), ds(WPK0 + WPK_W21E, C)]
            b1_sb = [bias_sb[:, ds(0, 1)], bias_sb[ds(0, 22), ds(2, 1)]]
            bm_sb = [bias_sb[:, ds(1, 1)], bias_sb[ds(0, 22), ds(3, 1)]]
            oh_off = (OH0, OH1)
            ft_off = (FT0, FT1)

            # PE p-state warmup during the prologue + feature-DMA window.
            # The operand is the runtime-preloaded [128,1] bf16 constant, so
            # the PE's first instruction has ZERO cross-engine dependencies
            # and starts the moment its queue opens. The HAM raises the PE
            # clock only at its free-running ~10.24us epoch boundaries IF the
            # PE was active, so every early busy ns raises the whole kernel's
            # clock; tiny N=1 matmuls give fine-grained adaptive coverage.
            dmy = constp.tile([128, 128], BF16, tag="dmy")
            nc.gpsimd.memset(dmy, 0.0)
            wts = [opps.tile([128, 2, C], F32, tag="op", name=f"warm{i}")
                   for i in range(2)]
            for i in range(NWARM):
                nc.tensor.matmul(wts[i % 2][:, 0, ds(0, 128)], dmy, dmy,
                                 start=True, stop=True)

            def diffT_stage(sg):
                """dp[hc][h, (r8,u)] = sum_tok feats[tok, h]*ohE4[tok, col];
                feats chunk h-slice stationary, ohE4 moving (N=224).
                hc-major so each 2-hc bank is evicted while later hc still
                run; casts alternate vector/scalar."""
                n = (nc0, nc1)[sg]
                diff = diffp.tile([128, HC, NU8], BF16, tag="diff")
                for hp in range(3):
                    dp = dpsum.tile([128, 2, NU8], F32, tag="dp")
                    for k in range(2):
                        hc = 2 * hp + k
                        for c in range(n):
                            nc.tensor.matmul(
                                dp[:, k, :],
                                in0[:, ds(ft_off[sg] + c * H + hc * 128, 128)],
                                in0[:, ds(oh_off[sg] + c * NU8, NU8)],
                                start=(c == 0), stop=(c == n - 1))
                    dst = diff[:, ds(2 * hp, 2), :]
                    # gpsimd cannot read PSUM; rotate vector/scalar
                    if hp == 1:
                        nc.scalar.activation(dst, dp, COPY)
                    else:
                        nc.vector.tensor_copy(dst, dp)
                return diff

            def mm1(sg, diff):
                """y = W1^T diffT (accumulate over hc); h1 = [relu(y+b1),
                relu(-y+b1)] (the +/- trick: one product, both pair
                orders). ci0 -> vector, minus branch -> scalar."""
                hp = h1ps.tile([128, 2 * NU8], F32, tag="h1p")
                h1 = []
                for ci, (coff, csz) in enumerate(CC):
                    out_ap = hp[ds(0, csz), ds(ci * NU8, NU8)]
                    for hc in range(HC):
                        nc.tensor.matmul(
                            out_ap,
                            w1_sb[:, ds(hc * C + coff, csz)],
                            diff[:, hc, :],
                            start=(hc == 0), stop=(hc == HC - 1))
                for ci, (coff, csz) in enumerate(CC):
                    src = hp[ds(0, csz), ds(ci * NU8, NU8)]
                    hs = actp.tile([csz, 2 * NU8], BF16, tag=f"h1_{ci}")
                    nc.vector.tensor_scalar(hs[:, ds(0, NU8)], src,
                                            b1_sb[ci], 0.0, ADD, MAX)
                    nc.scalar.activation(hs[:, ds(NU8, NU8)], src, RELU,
                                         bias=b1_sb[ci], scale=-1.0)
                    h1.append(hs)
                return h1

            def mm2(sg, h1):
                h2 = []
                for ci, (coff, csz) in enumerate(CC):
                    hp2 = (h2ps0, h2ps1)[ci].tile([csz, 2 * NU8], F32,
                                                  tag=f"h2p{ci}")
                    nc.tensor.matmul(hp2, wm0_sb[:, ds(coff, csz)], h1[0],
                                     start=True, stop=False)
                    nc.tensor.matmul(hp2, wm1_sb[:, ds(coff, csz)], h1[1],
                                     start=False, stop=True)
                    hs = actp.tile([csz + (1 if ci == 1 else 0), 2 * NU8],
                                   BF16, tag=f"h2_{ci}")
                    if ci == 0:
                        nc.scalar.activation(hs, hp2, RELU, bias=bm_sb[0])
                    else:
                        # row 22 stays 1.0 to pair with w21e's b2 row in
                        # mm3; partition-22 start is not a legal AP, so
                        # memset the whole tile then overwrite rows 0-21
                        nc.vector.memset(hs, 1.0)
                        nc.vector.tensor_scalar(hs[ds(0, 22), :], hp2,
                                                bm_sb[1], 0.0, ADD, MAX)
                    h2.append(hs)
                return h2

            def mm3(sg, h2, last=False):
                # evictions split vector/scalar: each op bank is freed
                # after ~one act latency, so the next tile's matmuls (and
                # SG1's bank reuse) never stall long. One out DMA per SG
                # (a DMAHW lane is an end-barrier cost on every engine).
                osb = osbp.tile([112, 4, C], F32, tag="osb")
                for t in range(2):
                    op = opps.tile([128, 2, C], F32, tag="op")
                    for sl in range(2):
                        s = t * 2 + sl
                        nc.tensor.matmul(op[ds(0, 112), sl, :],
                                         h2[0][:, ds(s * 112, 112)],
                                         w20_sb, start=True, stop=False)
                        nc.tensor.matmul(op[ds(0, 112), sl, :],
                                         h2[1][:, ds(s * 112, 112)],
                                         w21e_sb, start=False, stop=True)
                        if sl == 0:
                            nc.vector.tensor_copy(osb[:, s, :],
                                                  op[ds(0, 112), sl, :])
                        else:
                            nc.scalar.activation(osb[:, s, :],
                                                 op[ds(0, 112), sl, :], COPY)
                deng = nc.scalar if last else nc.sync
                deng.dma_start(
                    out=out_d[sg].rearrange("p (a c) -> p a c", a=4),
                    in_=osb)

            # 2-deep software pipeline: SG1's diffT fills the PE while
            # SG0's casts/activations run on vector/scalar/gpsimd.
            d0 = diffT_stage(0)
            nc.scalar.dma_start(out=in0[:, ds(WPK0, WPK_COLS)],
                                in_=blob_d[:, ds(WPK0, WPK_COLS)])
            d1 = diffT_stage(1)
            h1_0 = mm1(0, d0)
            h1_1 = mm1(1, d1)
            h2_0 = mm2(0, h1_0)
            h2_1 = mm2(1, h1_1)
            mm3(0, h2_0)
            mm3(1, h2_1, last=True)

    # TRN2 allows at most 1 sync wait per instruction (2 on event
    # semaphores); split the tile-emitted multi-waits like Bacc.compile().
    import bass_rust as _bass_rust
    _bass_rust.move_matmul_waits_to_ldweights(nc.m)
    _bass_rust.generate_event_semaphores(nc)
    return nc


def host_prep(output_ids, features, W1, b1, Wm, bm, W2, b2):
    ids = np.asarray(output_ids)
    B_, S_ = ids.shape
    feats = np.asarray(features)
    is_sep = ids == SEP_ID
    seg = np.cumsum(is_sep.astype(np.int64), axis=1)
    valid = (~is_sep) & (seg < NSEG)
    counts = np.stack([((seg == n) & valid).sum(1) for n in range(NSEG)],
                      axis=1).astype(np.float32)
    inv_c = (1.0 / np.maximum(counts, 1.0)).astype(np.float32)
    ntok = valid.sum(1)

    # per-row [8, 28] template: token in segment s contributes row s
    tmpl = np.zeros((B_, NSEG, NU), np.float32)
    for u in range(NU):
        tmpl[:, UI[u], u] += inv_c[:, UI[u]]
        tmpl[:, UJ[u], u] -= inv_c[:, UJ[u]]

    # snake-deal rows (sorted by token count) into 16 bins of 8
    nbins = NCORES * NSG
    rsg = B_ // nbins
    order = np.argsort(-ntok, kind="stable")
    bins = [[] for _ in range(nbins)]
    for rnd in range(rsg):
        chunk = order[rnd * nbins:(rnd + 1) * nbins]
        tgt = range(nbins) if rnd % 2 == 0 else range(nbins - 1, -1, -1)
        for t, bb in zip(tgt, chunk):
            bins[t].append(int(bb))
    Tbin = [int(sum(ntok[bb] for bb in bins[k])) for k in range(nbins)]
    ncs = [max(1, -(-Tbin[k] // 128)) for k in range(nbins)]
    NC = [max(ncs[sg * NCORES:(sg + 1) * NCORES]) for sg in range(NSG)]
    nc0, nc1 = NC
    NCT = nc0 + nc1

    W1 = np.asarray(W1, np.float32)
    Wm = np.asarray(Wm, np.float32)
    W2 = np.asarray(W2, np.float32)
    b1 = np.asarray(b1, np.float32)
    bm = np.asarray(bm, np.float32)
    b2 = np.asarray(b2, np.float32)

    wpk = np.zeros((128, WPK_COLS), np.float32)
    wpk[:, WPK_W1:WPK_W1 + HC * C] = (
        W1.reshape(HC, 128, C).transpose(1, 0, 2).reshape(128, HC * C))
    wpk[:, WPK_WM0:WPK_WM0 + C] = Wm[:128]
    wpk[:, WPK_W20:WPK_W20 + C] = W2[:128]
    wpk[:22, WPK_WM1:WPK_WM1 + C] = Wm[128:]
    wpk[:22, WPK_W21E:WPK_W21E + C] = W2[128:]
    wpk[22, WPK_W21E:WPK_W21E + C] = b2
    biasp = np.zeros((128, 4), np.float32)
    biasp[:, 0] = b1[:128]
    biasp[:, 1] = bm[:128]
    biasp[:22, 2] = b1[128:]
    biasp[:22, 3] = bm[128:]
    wpk = wpk.astype(NPBF16)
    # blob column offsets (must match build_program)
    OH0, FT0 = 0, nc0 * NU8
    OH1 = nc0 * (NU8 + H)
    FT1 = OH1 + nc1 * NU8
    WPK0 = OH1 + nc1 * (NU8 + H)
    BLOBCOLS = WPK0 + WPK_COLS

    # diagonal f(0) is input-independent: exact fp32 on host
    y0 = np.maximum(b1, 0.0)
    y1 = np.maximum(y0 @ Wm + bm, 0.0)
    diag = (y1 @ W2 + b2).astype(np.float32)

    in_maps, gather_maps = [], []
    for core in range(NCORES):
        blob = np.zeros((128, BLOBCOLS), NPBF16)
        blob[:, WPK0:WPK0 + WPK_COLS] = wpk
        for sg, (ncsg, ohoff, ftoff) in enumerate(
                ((nc0, OH0, FT0), (nc1, OH1, FT1))):
            fp = np.zeros((ncsg, 128, H), NPBF16)
            ohe = np.zeros((ncsg, 128, NU8), np.float32)
            rows = bins[sg * NCORES + core]
            pos = 0
            for r8, bb in enumerate(rows):
                toks = np.nonzero(valid[bb])[0]
                n = len(toks)
                if n == 0:
                    continue
                sl = np.arange(pos, pos + n)
                fp[sl // 128, sl % 128, :] = feats[bb, toks, :].astype(NPBF16)
                ohe[sl // 128, sl % 128, r8 * NU:(r8 + 1) * NU] = (
                    tmpl[bb, seg[bb, toks], :])
                pos += n
            blob[:, ohoff:ohoff + ncsg * NU8] = (
                ohe.transpose(1, 0, 2).reshape(128, ncsg * NU8))
            blob[:, ftoff:ftoff + ncsg * H] = (
                fp.transpose(1, 0, 2).reshape(128, ncsg * H))
        in_maps.append(dict(blob=np.ascontiguousarray(blob),
                            biasp=np.ascontiguousarray(biasp)))

        di, dj, db = [], [], []
        for sg in range(NSG):
            rows = bins[sg * NCORES + core]
            for half in range(2):
                for r8 in range(rsg):
                    bb = rows[r8]
                    for u in range(NU):
                        i, j = (UI[u], UJ[u]) if half == 0 else (UJ[u], UI[u])
                        di.append(i)
                        dj.append(j)
                        db.append(bb)
        gather_maps.append((np.array(di), np.array(dj), np.array(db)))

    aux = dict(nc0=nc0, nc1=nc1, gather_maps=gather_maps, diag=diag, B=B_)
    return in_maps, aux


def gather_output(core_outs, aux):
    full = np.empty((NSEG, NSEG, aux["B"], C), np.float32)
    for i in range(NSEG):
        full[i, i, :, :] = aux["diag"][None, :]
    for core, o in enumerate(core_outs):
        # device layout [NSG, 112, 4*C]: (sg, r, (s, c)), d = sg*448+s*112+r
        o = o.reshape(NSG, 112, 4, C).transpose(0, 2, 1, 3)
        o = np.ascontiguousarray(o).reshape(NSG * 2 * NU8, C)
        di, dj, db = aux["gather_maps"][core]
        full[di, dj, db, :] = o
    return full


_NC_CACHE = {}


def run(inputs, trace=False, trace_cores=None):
    in_maps, aux = host_prep(**inputs)
    key = (aux["nc0"], aux["nc1"])
    if key not in _NC_CACHE:
        _NC_CACHE[key] = build_program(*key)
    nc = _NC_CACHE[key]
    res = run_bass_kernel_spmd(
        nc, in_maps, core_ids=list(range(NCORES)),
        trace=trace, trace_cores=trace_cores,
    )
    out = gather_output([r["out"] for r in res.results], aux)
    return out, res


def kernel(**inputs):
    out, _ = run(inputs, trace=False)
    return out
